# revision 52
# baseline (speedup 1.0000x reference)
"""CANet (channel-attention net) Trainium2 kernel, v3.

8-core data parallel: 2 samples per core. Full inputs in, full output out.

Everything downstream of the convs is linear (concat -> mean -> FC), so all
convs whose outputs feed ONLY the pooled path are folded into host-side
weights contracted against cheap on-chip sums:
  - conv4/down4: logits += wfA @ (sum_n out4) + host(wfX @ sum_n x4)
  - conv3:  pooled(conv3(out3)) = wq3 . out3q where out3q[c,Q] are 9
    parity/strip sums of out3 = gamma*(K@X3S)/z + X3S, X3S hosted.
  - conv2b: pooled(conv2b(a2)) = wq2b . Q2b, Q2b = 15 parity/strip sums
    of the conv2a PSUM slabs (a2 never materialized).
Only conv2a runs spatially (fp8 DoubleRow, dual-weight error feedback).

CAM stages use PER-ROW exp bias (subsampled row-min) so the exp reads the
energy PSUM directly, pipelined with the energy matmuls (no E tile, no
PSUM->SBUF copies).  The per-row scale e^{b_row} is folded into the att
rhs (XS' = XS * e^{C-b}) and epilogue (gz *= e^{b-C}) around a global
midpoint C so everything stays in f32 range.  cam3/cam4 attention shrinks
to K @ XS (NQ=9 / NQ=1 columns).  cam2 keeps full spatial attention:
exact per-row bias, fp8 K, PE-transposed lhsT tiles, fp8 DoubleRow att.
"""

import numpy as np

import bass_rust
import concourse.bass as bass
import concourse.mybir as mybir
import concourse.tile as tile
from concourse.bass_utils import run_bass_kernel_spmd
from concourse.tile import ScopedClock

F32 = mybir.dt.float32
F32R = mybir.dt.float32r
BF16 = mybir.dt.bfloat16
F8 = mybir.dt.float8e4
AX = mybir.AxisListType
OP = mybir.AluOpType
AF = mybir.ActivationFunctionType
DR = mybir.MatmulPerfMode.DoubleRow

P = 128
NCORES = 8
S = 2


# ---------------------------------------------------------------------------
# walrus in this container supports at most ONE sync-wait per instruction;
# split extras across NOPs (tail drain + scheduled instructions).
# ---------------------------------------------------------------------------
_wsplit_counter = [0]


def _fresh_name(base):
    _wsplit_counter[0] += 1
    return f"{base}-wsplit{_wsplit_counter[0]}"


def _patched_drain_and_barrier(self, tick_clock, wait_clock):
    drain_inst = self.nc.sync.drain()
    wait_clock.add_sem_waits(
        drain_inst.ins, ScopedClock({None: tick_clock.global_clock})
    )
    ins = drain_inst.ins
    si = ins.sync_info
    waits = list(si.on_wait) if si is not None else []
    if len(waits) > 1:
        ins.sync_info = bass_rust.SyncInfo(on_wait=waits[:1], on_update=[])
        for w in waits[1:]:
            nop = self.nc.sync.nop(nofuse=True, hint="tail_wait_split")
            nop.ins.sync_info = bass_rust.SyncInfo(on_wait=[w], on_update=[])
    self.nc.all_engine_barrier()
    assert self.sems is not None
    popped = self.nc._tile_sem_poison_stack.pop()
    assert popped is self._sem_poison
    self.nc.clear_and_free_semaphores(list(self.sems.allocated().values()))
    self.nc.all_engine_barrier()


_orig_add_instruction = tile.TileContext._add_instruction


def _split_add_instruction(self, inst):
    si = inst.sync_info
    if si is not None:
        waits = list(si.on_wait)
        if len(waits) > 1:
            for w in waits[:-1]:
                nop = mybir.InstNoOp(name=_fresh_name(inst.name), ins=[], outs=[])
                nop.engine = inst.engine
                nop.sync_info = bass_rust.SyncInfo(on_wait=[w], on_update=[])
                _orig_add_instruction(self, nop)
            inst.sync_info = bass_rust.SyncInfo(
                on_wait=waits[-1:], on_update=list(si.on_update)
            )
    _orig_add_instruction(self, inst)


def _install_tilefix():
    tile.TileContext._drain_and_barrier = _patched_drain_and_barrier
    tile.TileContext._add_instruction = _split_add_instruction


def _f32(ap):
    return ap.bitcast(F32)


def _pe_warmup(nc, ones_row, bank, n=1):
    for _ in range(n):
        nc.tensor.matmul(bank[:, 0:128], _f32(ones_row[:]),
                         _f32(ones_row[:]), start=True, stop=True)


PHASE_MARKS = []


def _mark(nc, name):
    PHASE_MARKS.append((name, nc.next_id()))


def _cam_mini(nc, *, name, s, y_tile, XS, XSp, K, PS, gamma_b, out_q,
              small, C, NQ, ident, ones_row, ones_col):
    """cam3/cam4 stage: energy -> exp-from-psum (per-row subsampled bias)
    pipelined, then global midpoint C, XS' fold, tiny att, epilogue.

    y_tile: [P, KT, C] f32r.  XS: [P, MT, NQ] hosted sums (f32).
    XSp: [P, MT, NQ] scratch for scaled XS'.  K: [P, MT, C] f32r.
    out_q: [P, S, MT, NQ] f32 output sums.
    """
    MT = C // P
    KT = {2048: 2, 1024: 8}[C]
    NCH = C // 512
    GRP = 8 // NCH  # psum tile groups in flight

    tminP = small.tile([P, MT], F32, tag=f"tminP_{name}", name="tminP")
    runx = small.tile([P, 2], F32R, tag=f"runx_{name}", name="runx")

    for m in range(MT):
        base = (m % GRP) * NCH
        hmin = small.tile([P, NCH], F32, tag=f"hmin_{name}", bufs=2,
                          name="hmin")
        for h in range(NCH):
            ps = PS[base + h]
            for k in range(KT):
                nc.tensor.matmul(
                    ps[:],
                    y_tile[:, k, m * P:(m + 1) * P],
                    y_tile[:, k, h * 512:(h + 1) * 512],
                    start=(k == 0), stop=(k == KT - 1),
                )
            nc.vector.tensor_reduce(
                hmin[:, h:h + 1], ps[:], axis=AX.X, op=OP.min)
        nc.vector.tensor_reduce(tminP[:, m:m + 1], hmin[:],
                                axis=AX.X, op=OP.min)
        # exp straight from PSUM with per-row bias (no accum: z comes
        # from an extra att column)
        for h in range(NCH):
            nc.scalar.activation(
                K[:, m, h * 512:(h + 1) * 512], PS[base + h][:],
                AF.Exp, bias=tminP[:, m:m + 1], scale=-1.0)
        ntmin = small.tile([P, 1], F32, tag=f"ntmin_{name}", bufs=2,
                           name="ntmin")
        nc.vector.tensor_scalar(ntmin[:], tminP[:, m:m + 1], -1.0, None,
                                op0=OP.mult)
        if m == 0:
            nc.vector.tensor_copy(runx[:, 0:1], tminP[:, 0:1])
            nc.vector.tensor_copy(runx[:, 1:2], ntmin[:])
        else:
            nc.vector.tensor_tensor(runx[:, 0:1], _f32(runx[:, 0:1]),
                                    tminP[:, m:m + 1], op=OP.max)
            nc.vector.tensor_tensor(runx[:, 1:2], _f32(runx[:, 1:2]),
                                    ntmin[:], op=OP.max)

    # --- global midpoint C via two tiny PE transposes (borrow banks) ---
    tpa = PS[0][0:2, 0:128]
    nc.tensor.transpose(tpa.bitcast(F32R), runx[:], ident[:])
    mm = small.tile([2, 4], F32R, tag=f"mm_{name}", name="mm")
    nc.vector.tensor_reduce(mm[:, 0:1], tpa, axis=AX.X, op=OP.max)
    nc.vector.tensor_copy(mm[:, 1:2], _f32(mm[:, 0:1]))
    nc.vector.tensor_copy(mm[:, 2:4], _f32(mm[:, 0:2]))
    tpb = PS[1][0:4, 0:4]
    nc.tensor.transpose(tpb.bitcast(F32R), mm[:], ident[0:2, 0:4])
    mm2s = small.tile([1, 4], F32, tag=f"mm2s_{name}", name="mm2s")
    nc.vector.tensor_copy(mm2s[:], tpb[0:1, 0:4])
    # C = (gmax + gmin)/2 = (mm2s[0] - mm2s[1]) / 2
    cc = small.tile([1, 1], F32, tag=f"cc_{name}", name="cc")
    nc.vector.tensor_tensor(cc[:], mm2s[:, 0:1], mm2s[:, 1:2],
                            op=OP.subtract)
    nc.vector.tensor_scalar(cc[:], cc[:], 0.5, None, op0=OP.mult)
    bps = PS[2][0:P, 0:1]
    nc.tensor.matmul(bps, _f32(ones_row[:]), cc[:], start=True, stop=True)
    cP = small.tile([P, 1], F32, tag=f"cP_{name}", name="cP")
    nc.vector.tensor_copy(cP[:], bps)

    # XS' = XS * e^{C-b}; extra column carries e^{C-b} itself so the att
    # matmul also produces z (the e^C factors cancel in gz = gamma/att_z).
    ef = small.tile([P, MT], F32, tag=f"ef_{name}", name="ef")
    nc.scalar.activation(ef[:], tminP[:], AF.Exp, bias=cP[:, 0:1],
                         scale=-1.0)
    for m in range(MT):
        nc.vector.tensor_scalar(XSp[:, m, 0:NQ], XS[:, m, :],
                                ef[:, m:m + 1], None, op0=OP.mult)
    nc.vector.tensor_copy(XSp[:, :, NQ], ef[:])
    # keep the PE clock ramped across the C-chain stall
    _pe_warmup(nc, ones_row, PS[4], n=4)

    # --- tiny attention: attps[m] = sum_j K[j, m-block]^T-sym @ XS'[j] ---
    NQ1 = NQ + 1
    attv = PS[3][:, 0:MT * NQ1].rearrange("p (m q) -> p m q", m=MT)
    for j in range(MT):
        for m in range(MT):
            nc.tensor.matmul(
                attv[:, m, :], K[:, j, m * P:(m + 1) * P], XSp[:, j, :],
                start=(j == 0), stop=(j == MT - 1))
    z = small.tile([P, MT], F32, tag=f"z_{name}", name="z")
    nc.vector.tensor_copy(z[:], attv[:, :, NQ])
    rz = small.tile([P, MT], F32, tag=f"rz_{name}", name="rz")
    nc.vector.reciprocal(rz[:], z[:])
    gz = small.tile([P, MT], F32, tag=f"gz_{name}", name="gz")
    nc.vector.tensor_scalar(gz[:], rz[:], gamma_b[:, 0:1], None, op0=OP.mult)
    for m in range(MT):
        nc.vector.scalar_tensor_tensor(
            out_q[:, s, m, :], attv[:, m, 0:NQ], gz[:, m:m + 1],
            XS[:, m, :], op0=OP.mult, op1=OP.add)


def build_program(debug=False):
    _install_tilefix()
    PHASE_MARKS.clear()
    nc = bass.Bass(name="canet", dynamic_dma_scratch_size=2048)

    y4_d = nc.dram_tensor("y4", [S, 256, 2048], F32R, kind="ExternalInput")
    y3_d = nc.dram_tensor("y3", [S, 1024, 1024], F32R, kind="ExternalInput")
    y2_d = nc.dram_tensor("y2", [S, 4096, 512], F32R, kind="ExternalInput")
    x2_d = nc.dram_tensor("x2", [S, 512, 4096], F8, kind="ExternalInput")
    x2b_d = nc.dram_tensor("x2b", [S, 512, 4096], BF16, kind="ExternalInput")
    x4s_d = nc.dram_tensor("x4s", [P, S, 16], F32, kind="ExternalInput")
    x3s_d = nc.dram_tensor("x3s", [S, 1024, 9], F32, kind="ExternalInput")
    w2a_d = nc.dram_tensor("w2a8", [9, 512, 512], F8, kind="ExternalInput")
    w2b_d = nc.dram_tensor("w2a8b", [9, 512, 512], F8, kind="ExternalInput")
    wfa_d = nc.dram_tensor("wfa", [2048, 2], F32, kind="ExternalInput")
    wq3_d = nc.dram_tensor("wq3", [1024, 9, 2], F32, kind="ExternalInput")
    wq2b_d = nc.dram_tensor("wq2b", [512, 15, 2], F32, kind="ExternalInput")
    hlog_d = nc.dram_tensor("hlog", [S, 2], F32, kind="ExternalInput")
    g2_d = nc.dram_tensor("g2", [1], F32, kind="ExternalInput")
    g3_d = nc.dram_tensor("g3", [1], F32, kind="ExternalInput")
    g4_d = nc.dram_tensor("g4", [1], F32, kind="ExternalInput")
    ident_d = nc.dram_tensor("ident", [P, 128], F32R, kind="ExternalInput")
    ident8_d = nc.dram_tensor("ident8", [P, 128], F8, kind="ExternalInput")
    out_d = nc.dram_tensor("out", [S, 2], F32, kind="ExternalOutput")

    with tile.TileContext(nc) as tc:
        with tc.tile_pool(name="persist", bufs=1) as persist, \
             tc.tile_pool(name="scratch", bufs=2) as scratch, \
             tc.tile_pool(name="gpsum", bufs=1, space="PSUM") as gpsum:

            PS = [gpsum.tile([P, 512], F32, tag=f"bank{i}", name=f"bank{i}")
                  for i in range(8)]

            ident = persist.tile([P, 128], F32R)
            ident8 = persist.tile([P, 128], F8)
            ones_row = persist.tile([1, 128], F32R)
            nc.vector.memset(_f32(ones_row[:]), 1.0)
            ones_col = persist.tile([P, 4], F32R)
            nc.vector.memset(_f32(ones_col[:]), 1.0)
            for w in range(10):
                nc.tensor.matmul(PS[7][:, 0:128],
                                 _f32(ones_row[:]), _f32(ones_row[:]),
                                 start=True, stop=True)
            gb = {}
            for nm in ("g2", "g3", "g4"):
                gb[nm] = persist.tile([P, 1], F32, tag=f"gb_{nm}",
                                      name=f"gb_{nm}")
            X4S = persist.tile([P, S, 16, 1], F32)
            X3S = persist.tile([P, S, 8, 9], F32)
            out4q = persist.tile([P, S, 16, 1], F32)
            out3q = persist.tile([P, S, 8, 9], F32)
            Q2b = persist.tile([P, S, 4, 15], F32)
            wfa = persist.tile([P, 16, 2], F32)
            wq3 = persist.tile([P, 8, 9, 2], F32)
            wq2b = persist.tile([P, 4, 15, 2], F32)
            hlog = persist.tile([2, S], F32)
            logacc = persist.tile([2, S], F32)
            nc.vector.memset(logacc[:], 0.0)
            XSp4 = persist.tile([P, 16, 2], F32R, name="XSp4")
            XSp3 = persist.tile([P, 8, 10], F32R, name="XSp3")

            def load_persist():
                nc.sync.dma_start(ident[:], ident_d[:])
                nc.sync.dma_start(ident8[:], ident8_d[:])
                for nm, gd in (("g2", g2_d), ("g3", g3_d), ("g4", g4_d)):
                    nc.sync.dma_start(gb[nm][:], gd[:].to_broadcast((P, 1)))
                nc.sync.dma_start(
                    X4S[:].rearrange("p s m q -> p s (m q)"), x4s_d[:])
                nc.sync.dma_start(
                    X3S[:], x3s_d[:].rearrange("s (m p) q -> p s m q", p=P))
                nc.sync.dma_start(wfa[:],
                                  wfa_d[:].rearrange("(m p) l -> p m l", p=P))
                nc.sync.dma_start(wq3[:],
                                  wq3_d[:].rearrange("(m p) q l -> p m q l",
                                                     p=P))
                nc.sync.dma_start(wq2b[:],
                                  wq2b_d[:].rearrange("(o p) q l -> p o q l",
                                                      p=P))
                nc.sync.dma_start(hlog[:], hlog_d[:].rearrange("s l -> l s"))

            # ======== CAM2 + conv2a (fp8) + folded conv2b ========
            with tc.tile_pool(name="cam2outer", bufs=1) as cam2outer, \
                 tc.tile_pool(name="c2apool", bufs=1) as c2apool, \
                 tc.tile_pool(name="cam2pool", bufs=1) as cam2pool:
                out2p = cam2outer.tile([P, 4, 66, 66], F8)
                K2 = cam2outer.tile([P, 4, 512], F8)
                PT2 = cam2outer.tile([P, 4, 512], F8)
                w2ar = cam2outer.tile([P, 2, 9, 4, 512], F8)
                nc.gpsimd.memset(out2p[:, :, 0:66:65, :], 0.0)
                nc.gpsimd.memset(out2p[:, :, 1:65, 0:66:65], 0.0)
                for s in range(S):
                    _mark(nc, f"cam2_s{s}")
                    mins = scratch.tile([P, 4], F32, tag="mins2",
                                        name="mins2")
                    for k in range(32):
                        yk = cam2pool.tile([P, 512], F32R, tag="ystream",
                                           bufs=8, name="yk")
                        nc.sync.dma_start(yk[:],
                                          y2_d[s, k * P:(k + 1) * P, :])
                        for m in range(4):
                            nc.tensor.matmul(
                                PS[m][:], yk[:, m * P:(m + 1) * P],
                                yk[:], start=(k == 0), stop=(k == 31))
                    if s == 0:
                        load_persist()
                    z2 = scratch.tile([P, 4], F32, tag="z_c2", name="z2")
                    for m in range(4):
                        nc.vector.tensor_reduce(
                            mins[:, m:m + 1], PS[m][:], axis=AX.X, op=OP.min)
                        nc.scalar.activation(
                            K2[:, m, :], PS[m][:], AF.Exp,
                            bias=mins[:, m:m + 1], scale=-1.0)
                        # z from the QUANTIZED K2 so normalization is
                        # self-consistent under any fp8 rounding mode
                        nc.vector.tensor_reduce(
                            z2[:, m:m + 1], K2[:, m, :], axis=AX.X,
                            op=OP.add)
                    rz2 = scratch.tile([P, 4], F32, tag="rz_c2", name="rz2")
                    nc.vector.reciprocal(rz2[:], z2[:])
                    gz2 = scratch.tile([P, 4], F32, tag="gz_c2", name="gz2")
                    nc.vector.tensor_scalar(gz2[:], rz2[:], gb["g2"][:, 0:1],
                                            None, op0=OP.mult)

                    # PT2 = K2^T via fp8 PE transposes of 128x128 blocks
                    # (fp8 transpose writes with element step 2 in PSUM)
                    for i in range(4):
                        for j in range(4):
                            tp = PS[4 + ((i * 4 + j) % 2)][:, 0:64]
                            tp8 = tp.bitcast(F8)[:, 0:256:2]
                            nc.tensor.transpose(
                                tp8, K2[:, i, j * P:(j + 1) * P], ident8[:])
                            if (i * 4 + j) % 2:
                                nc.vector.tensor_copy(
                                    PT2[:, j, i * P:(i + 1) * P], tp8)
                            else:
                                nc.scalar.copy(
                                    PT2[:, j, i * P:(i + 1) * P], tp8)

                    # att @ x2 fp8 DoubleRow, 8 column chunks of 512
                    for c in range(8):
                        xc = cam2pool.tile([P, 4, 512], F8, tag="x2c",
                                           bufs=4, name="x2c")
                        nc.sync.dma_start(
                            xc[:],
                            x2_d[s, :, c * 512:(c + 1) * 512].rearrange(
                                "(j p) n -> p j n", p=P))
                        # bf16 copy feeds the residual (avoids double fp8
                        # rounding of x2 in out2p)
                        xcb = cam2pool.tile([P, 4, 512], BF16, tag="x2cb",
                                            bufs=4, name="x2cb")
                        nc.sync.dma_start(
                            xcb[:],
                            x2b_d[s, :, c * 512:(c + 1) * 512].rearrange(
                                "(j p) n -> p j n", p=P))
                        if s == 0 and c < 2:
                            # prime conv2a tap 0/1 during the att phase
                            nc.sync.dma_start(
                                w2ar[:, 0, c],
                                w2a_d[c].rearrange("(k p) o -> p k o", p=P))
                            nc.sync.dma_start(
                                w2ar[:, 1, c],
                                w2b_d[c].rearrange("(k p) o -> p k o", p=P))
                        base = (c % 2) * 4
                        for m in range(4):
                            ps = PS[base + m]
                            for jp in range(2):
                                nc.tensor.matmul(
                                    ps[:],
                                    PT2[:, 2 * jp:2 * jp + 2,
                                        m * P:(m + 1) * P],
                                    xc[:, 2 * jp:2 * jp + 2, :],
                                    start=(jp == 0), stop=(jp == 1),
                                    perf_mode=DR)
                            nc.vector.scalar_tensor_tensor(
                                out2p[:, m, 1 + 8 * c:9 + 8 * c, 1:65],
                                ps[:].rearrange("p (a b) -> p a b", a=8),
                                gz2[:, m:m + 1],
                                xc[:, m, :].rearrange("p (a b) -> p a b",
                                                      a=8),
                                op0=OP.mult, op1=OP.add)

                    # conv2a: fp8 DoubleRow, dual-weight error feedback
                    _mark(nc, f"conv2a_s{s}")
                    vpa = [PS[i][:].rearrange("p (a b) -> p a b", a=32)
                           for i in range(8)]
                    for t9 in range(9):
                        ky, kx = t9 // 3, t9 % 3
                        if s == 0 and t9 < 7:
                            # stream tap t9+2 while computing tap t9
                            nc.sync.dma_start(
                                w2ar[:, 0, t9 + 2],
                                w2a_d[t9 + 2].rearrange("(k p) o -> p k o",
                                                        p=P))
                            nc.sync.dma_start(
                                w2ar[:, 1, t9 + 2],
                                w2b_d[t9 + 2].rearrange("(k p) o -> p k o",
                                                        p=P))
                        for o in range(4):
                            for h in range(2):
                                for ci in range(2):
                                    for kp in range(2):
                                        rhs = out2p[
                                            :, 2 * kp:2 * kp + 2,
                                            ky:ky + 64:2,
                                            kx + 32 * h:kx + 32 * h + 32:2]
                                        nc.tensor.matmul(
                                            vpa[o * 2 + h],
                                            w2ar[:, ci, t9,
                                                 2 * kp:2 * kp + 2,
                                                 o * P:(o + 1) * P],
                                            rhs,
                                            start=(t9 == 0 and ci == 0
                                                   and kp == 0),
                                            stop=(t9 == 8 and ci == 1
                                                  and kp == 1),
                                            perf_mode=DR)
                    # Q2b: 15 parity/strip sums per o-chunk from PSUM slabs
                    _mark(nc, f"q2b_s{s}")
                    for o in range(4):
                        for h in range(2):
                            v = vpa[o * 2 + h]
                            qb = 6 * h  # h0 cols 0-5, h1 cols 6-14
                            for qi, (rp, cp) in enumerate(
                                    ((0, 0), (0, 1), (1, 0), (1, 1))):
                                if qi in (1, 2):
                                    # Act reduce via accum-copy
                                    qscr = scratch.tile(
                                        [P, 16, 8], F32, tag="qscr",
                                        bufs=2, name="qscr")
                                    nc.scalar.activation(
                                        qscr[:], v[:, rp::2, cp::2],
                                        AF.Identity,
                                        accum_out=Q2b[:, s, o,
                                                      qb + qi:qb + qi + 1])
                                else:
                                    nc.vector.tensor_reduce(
                                        Q2b[:, s, o, qb + qi:qb + qi + 1],
                                        v[:, rp::2, cp::2], axis=AX.XY,
                                        op=OP.add)
                            for cp in range(2):
                                nc.vector.tensor_reduce(
                                    Q2b[:, s, o, qb + 4 + cp:qb + 5 + cp],
                                    v[:, 31, cp::2], axis=AX.X,
                                    op=OP.add)
                            if h == 1:
                                for rp in range(2):
                                    nc.vector.tensor_reduce(
                                        Q2b[:, s, o, 12 + rp:13 + rp],
                                        v[:, rp::2, 15:16], axis=AX.XY,
                                        op=OP.add)
                                nc.vector.tensor_copy(
                                    Q2b[:, s, o, 14:15], v[:, 31, 15:16])

                _mark(nc, "s2bfold")
                ps2 = PS[6][0:2, 264:266]
                for o in range(4):
                    for q in range(15):
                        nc.tensor.matmul(ps2, wq2b[:, o, q, :],
                                         Q2b[:, :, o, q],
                                         start=(o == 0 and q == 0),
                                         stop=(o == 3 and q == 14))
                nc.vector.tensor_tensor(logacc[:], logacc[:], ps2,
                                        op=OP.add)

            # ================= CAM3 (+ folded conv3) =================
            with tc.tile_pool(name="cam3pool", bufs=1) as cam3pool:
                K3 = cam3pool.tile([P, 8, 1024], F32R)
                for s in range(S):
                    _mark(nc, f"cam3_s{s}")
                    _pe_warmup(nc, ones_row, PS[5], n=4)
                    y3 = persist.tile([P, 8, 1024], F32R, tag="y3", bufs=1,
                                      name="y3")
                    for k in range(8):
                        nc.sync.dma_start(y3[:, k],
                                          y3_d[s, k * P:(k + 1) * P, :])
                    _cam_mini(nc, name="c3", s=s, y_tile=y3,
                              XS=X3S[:, s], XSp=XSp3, K=K3, PS=PS,
                              gamma_b=gb["g3"], out_q=out3q, small=scratch,
                              C=1024, NQ=9, ident=ident, ones_row=ones_row,
                              ones_col=ones_col)
                _mark(nc, "s3fold")
                ps3 = PS[6][0:2, 260:262]
                for m in range(8):
                    for q in range(9):
                        nc.tensor.matmul(ps3, wq3[:, m, q, :],
                                         out3q[:, :, m, q],
                                         start=(m == 0 and q == 0),
                                         stop=(m == 7 and q == 8))
                nc.vector.tensor_tensor(logacc[:], logacc[:], ps3, op=OP.add)

            # ================= CAM4 (+ folded conv4) =================
            with tc.tile_pool(name="cam4pool", bufs=1) as cam4pool:
                K4 = cam4pool.tile([P, 16, 2048], F32R)
                for s in range(S):
                    _mark(nc, f"cam4_s{s}")
                    _pe_warmup(nc, ones_row, PS[5], n=4)
                    y4 = persist.tile([P, 2, 2048], F32R, tag="y4", bufs=2,
                                      name="y4")
                    for k in range(2):
                        nc.sync.dma_start(y4[:, k],
                                          y4_d[s, k * P:(k + 1) * P, :])
                    _cam_mini(nc, name="c4", s=s, y_tile=y4,
                              XS=X4S[:, s], XSp=XSp4, K=K4, PS=PS,
                              gamma_b=gb["g4"], out_q=out4q, small=scratch,
                              C=2048, NQ=1, ident=ident, ones_row=ones_row,
                              ones_col=ones_col)
                # A-fold: logits += wfa . out4q
                _mark(nc, "afold")
                psA = PS[6][0:2, 256:258]
                for m in range(16):
                    nc.tensor.matmul(psA, wfa[:, m, :],
                                     out4q[:, :, m, 0],
                                     start=(m == 0), stop=(m == 15))
                nc.vector.tensor_tensor(logacc[:], logacc[:], psA, op=OP.add)

            # ================= output =================
            _mark(nc, "out")
            fc_o = scratch.tile([2, S], F32, tag="fc_o", name="fc_o")
            nc.vector.tensor_tensor(fc_o[:], logacc[:], hlog[:], op=OP.add)
            nc.sync.dma_start(out_d[:].rearrange("s l -> l s"), fc_o[:])

    return nc


def prepare_in_maps(c2, c3, c4, w4, b4, w3, w2a, w2b, b2b, g2, g3, g4,
                    fc_w, fc_b):
    import ml_dtypes
    E4M = ml_dtypes.float8_e4m3
    f32 = np.float32
    B = c2.shape[0]
    BFD = ml_dtypes.bfloat16
    c2f = np.ascontiguousarray(c2, dtype=f32).reshape(B, 512, 4096)
    c3f = np.ascontiguousarray(c3, dtype=f32).reshape(B, 1024, 1024)
    c4f = np.ascontiguousarray(c4, dtype=f32).reshape(B, 2048, 256)
    y2 = np.ascontiguousarray(c2f.transpose(0, 2, 1))
    y3 = np.ascontiguousarray(c3f.transpose(0, 2, 1))
    y4 = np.ascontiguousarray(c4f.transpose(0, 2, 1))
    x2_8 = np.ascontiguousarray(c2f).astype(E4M)

    X4S = c4f.sum(-1)                                   # [B, 2048]
    x3sp = c3f.reshape(B, 1024, 32, 32)
    q = []
    for rp in range(2):
        for cp in range(2):
            q.append(x3sp[:, :, rp::2, cp::2].sum((-1, -2)))
    q.append(x3sp[:, :, 31, 0::2].sum(-1))
    q.append(x3sp[:, :, 31, 1::2].sum(-1))
    q.append(x3sp[:, :, 0::2, 31].sum(-1))
    q.append(x3sp[:, :, 1::2, 31].sum(-1))
    q.append(x3sp[:, :, 31, 31])
    X3S = np.stack(q, axis=-1).astype(f32)              # [B, 1024, 9]

    w4m = np.asarray(w4, f32)[:, :, 0, 0]               # [512, 2048]
    fcw2 = np.asarray(fc_w, f32)[:, 0:512]
    fcw3 = np.asarray(fc_w, f32)[:, 512:1024]
    fcw4 = np.asarray(fc_w, f32)[:, 1024:1536]
    fcwc4 = np.asarray(fc_w, f32)[:, 1536:3584]
    wfa = (fcw4 @ w4m / 256.0).T.astype(f32)            # [2048, 2]
    wfx = ((fcw2 + fcw3 + fcw4) @ w4m + fcwc4) / 256.0  # [2, 2048]
    pbias = np.concatenate([np.asarray(b4, f32) + np.asarray(b2b, f32),
                            np.asarray(b4, f32), 2.0 * np.asarray(b4, f32),
                            np.zeros(2048, f32)])
    fcbp = np.asarray(fc_b, f32) + np.asarray(fc_w, f32) @ pbias
    hlog = (X4S @ wfx.T + fcbp[None, :]).astype(f32)    # [B, 2]

    # wq3[c, Q, l]: S3 inclusion-exclusion folded into fc.
    # Q order: PP(ee,eo,oe,oo), R31(e,o), C31(e,o), corner.
    # rp(ky): ky=0->odd(1), ky=1->even(0), ky=2->odd(1); same for kx.
    rsel = {0: [1], 1: [0], 2: [1]}
    w3m = np.asarray(w3, f32)                           # [512,1024,3,3]
    wf3 = np.einsum('lo,ocyx->lcyx', fcw3, w3m) / 256.0  # [2,1024,3,3]

    def fold_q(wf):
        L, C = wf.shape[0], wf.shape[1]
        out = np.zeros((C, 9, L), f32)
        for ky in range(3):
            for kx in range(3):
                rp, cp = rsel[ky][0], rsel[kx][0]
                out[:, rp * 2 + cp, :] += wf[:, :, ky, kx].T
        for kx in range(3):
            cp = rsel[kx][0]
            out[:, 4 + cp, :] -= wf[:, :, 0, kx].T
        for ky in range(3):
            rp = rsel[ky][0]
            out[:, 6 + rp, :] -= wf[:, :, ky, 0].T
        out[:, 8, :] += wf[:, :, 0, 0].T
        return out

    wq3 = fold_q(wf3)                                   # [1024, 9, 2]
    w2bm = np.asarray(w2b, f32)
    wf2b = np.einsum('lo,ocyx->lcyx', fcw2, w2bm) / (256.0 * 128.0)
    wq2b9 = fold_q(wf2b)                                # [512, 9, 2]
    # expand to 15 raw partials: h0 [PPx4, R31x2], h1 [PPx4, R31x2,
    # C31x2, corner]; PP/R31 shared across h.
    wq2b = np.zeros((512, 15, 2), f32)
    wq2b[:, 0:4] = wq2b9[:, 0:4]
    wq2b[:, 4:6] = wq2b9[:, 4:6]
    wq2b[:, 6:10] = wq2b9[:, 0:4]
    wq2b[:, 10:12] = wq2b9[:, 4:6]
    wq2b[:, 12:14] = wq2b9[:, 6:8]
    wq2b[:, 14:15] = wq2b9[:, 8:9]

    w2at = np.ascontiguousarray(
        np.asarray(w2a, f32).transpose(2, 3, 1, 0).reshape(9, 512, 512))
    w2a8 = (w2at * 128.0).astype(E4M)
    w2a8b = (w2at * 128.0 - w2a8.astype(f32)).astype(E4M)

    in_maps = []
    for core in range(NCORES):
        s0 = core * S
        sl = slice(s0, s0 + S)
        in_maps.append({
            "y2": np.ascontiguousarray(y2[sl]),
            "y3": np.ascontiguousarray(y3[sl]),
            "y4": np.ascontiguousarray(y4[sl]),
            "x2": np.ascontiguousarray(x2_8[sl]),
            "x2b": np.ascontiguousarray(c2f[sl].astype(BFD)),
            "x4s": np.ascontiguousarray(
                X4S[sl].reshape(S, 16, P).transpose(2, 0, 1).astype(f32)),
            "x3s": np.ascontiguousarray(X3S[sl]),
            "w2a8": w2a8, "w2a8b": w2a8b,
            "wfa": np.ascontiguousarray(wfa),
            "wq3": np.ascontiguousarray(wq3),
            "wq2b": np.ascontiguousarray(wq2b),
            "hlog": np.ascontiguousarray(hlog[sl]),
            "g2": np.asarray(g2, f32), "g3": np.asarray(g3, f32),
            "g4": np.asarray(g4, f32),
            "ident": np.eye(128, dtype=f32),
            "ident8": np.eye(128, dtype=f32).astype(E4M),
        })
    return in_maps


def kernel(**inputs):
    nc = build_program()
    in_maps = prepare_in_maps(**inputs)
    res = run_bass_kernel_spmd(nc, in_maps, core_ids=list(range(NCORES)))
    out = np.concatenate([r["out"] for r in res.results], axis=0)
    return out.astype(np.float32)
